# revision 18
# baseline (speedup 1.0000x reference)
"""minLSTM (2-layer, B=4, S=4096, D=1024) on 8 Trainium2 NeuronCores.

Sharding: core k -> (batch b = k//2, channel half h = k%2).
Each core computes all 4096 timesteps for its batch and its 512 channels.

Key optimizations over the f32r baseline (629us):
  - per-gate matmul dtypes: i/f gates via fp8e4m3 DoubleRow matmuls (2 k-tiles
    per instruction = 2x PE throughput; the i'/f' normalization suppresses
    their quantization noise), o/cell gates in bf16 (error-sensitive paths).
    Weights scaled by 64 into fp8 range; descale rides the ACT `scale`.
  - the f'=sig(f)/(sig(f)+sig(i)) division moved off the DVE: the 3352ns
    DVE RECIPROCAL becomes r = Exp(-Ln(ssum)) on the ACT engine (ln and exp
    share one activation table; sigmoid phases and ln/exp phases are batched
    per token block so only 2 table loads per block are inserted).
  - elementwise ops run 1024-wide on j-chunk pairs (fixed op overhead halved),
    in bf16 where the error budget allows (2x DVE throughput).
  - recurrence via DVE tensor_tensor_scan in linear space, f32 internal state.
Between layers, channel-half pairs exchange h1 via pairwise AllGather in both
bf16 (for bf16-gate rhs) and fp8 (for DoubleRow rhs), overlapped with compute.

Self-contained: hardcodes shapes; only imports the system concourse repo.
"""
import os
import sys

if '/opt/trn_rl_repo' not in sys.path:
    sys.path.insert(0, '/opt/trn_rl_repo')

import numpy as np

B, S, D = 4, 4096, 1024
NCORES = 8
HALF = D // 2           # channels per core: 512
NCHUNK = HALF // 128    # 4 partition chunks of 128 channels
NKT = D // 128          # 8 contraction k-tiles
NKP = NKT // 2          # 4 contraction k-pairs (fp8 DoubleRow)
TBLK = 512              # token block
NBLK = S // TBLK        # 8 token blocks
GCH = 4 * HALF          # gate channels per core: 2048
SC = 64.0               # fp8 weight scale

QNAMES = ("i", "f", "o", "cell")
# which gate quarters run through fp8 DoubleRow matmuls
FP8_GATES = set(os.environ.get("MINLSTM_FP8", "if"))         # subset of {i,f,o,c}
FP8_GATES = {q for q in QNAMES if q[0] in FP8_GATES}
# which elementwise values are computed/stored in bf16
EW_BF16 = set(os.environ.get("MINLSTM_BF16", "h,so,sg,g,bt,a,sf").split(","))

_CACHE = {}


def _split_multi_waits(nc):
    """This walrus build rejects >1 sync wait per instruction. Hoist extra
    waits onto same-engine NoOps inserted just before; engine-queue program
    order makes this semantically identical."""
    from concourse import mybir
    n = 0
    for fn in nc.m.functions:
        for blk in fn.blocks:
            insts = list(blk.instructions)
            new = []
            changed = False
            for inst in insts:
                si = inst.sync_info
                ow = list(si.on_wait) if si is not None and si.on_wait else []
                if len(ow) > 1:
                    changed = True
                    for w in ow[:-1]:
                        n += 1
                        nop = mybir.InstNoOp(name=f"I-wsplit-{n}", ins=[], outs=[])
                        nop.engine = inst.engine
                        nop.sync_info = mybir.SyncInfo(on_wait=[w], on_update=[])
                        new.append(nop)
                    si.on_wait = [ow[-1]]
                new.append(inst)
            if changed:
                blk.instructions = new
    return n


def _build_nc():
    import concourse.bass as bass
    import concourse.mybir as mybir
    import concourse.tile as tile

    f32 = mybir.dt.float32
    bf16 = mybir.dt.bfloat16
    fp8 = mybir.dt.float8e4
    AF = mybir.ActivationFunctionType
    ALU = mybir.AluOpType
    DR = mybir.MatmulPerfMode.DoubleRow

    def ew(name):
        return bf16 if name in EW_BF16 else f32

    q8list = [q for q in QNAMES if q in FP8_GATES]
    qblist = [q for q in QNAMES if q not in FP8_GATES]
    C8 = len(q8list) * HALF      # fp8 weight columns per core
    C16 = len(qblist) * HALF     # bf16 weight columns per core
    use8 = C8 > 0
    use16 = C16 > 0

    nc = bass.Bass("TRN2", target_bir_lowering=False, debug=False,
                   num_devices=NCORES)

    x8_d = nc.dram_tensor("x8T", [D, S], fp8, kind="ExternalInput").ap() \
        if use8 else None
    xb_d = nc.dram_tensor("xbT", [D, S], bf16, kind="ExternalInput").ap() \
        if use16 else None
    w8_d = [nc.dram_tensor(f"w8_{l}", [NKP * 128, 2 * C8], fp8,
                           kind="ExternalInput").ap() if use8 else None
            for l in range(2)]
    w16_d = [nc.dram_tensor(f"w16_{l}", [NKT * 128, C16], bf16,
                            kind="ExternalInput").ap() if use16 else None
             for l in range(2)]
    ba_d = [nc.dram_tensor(f"b{l}a", [128, 16], f32, kind="ExternalInput").ap()
            for l in range(2)]
    bc_d = [nc.dram_tensor(f"b{l}c", [128, 4], f32, kind="ExternalInput").ap()
            for l in range(2)]
    cp_d = [nc.dram_tensor(f"cp{l}", [128, 4], f32, kind="ExternalInput").ap()
            for l in range(2)]
    h2t_d = nc.dram_tensor("h2t", [HALF, S], bf16, kind="ExternalOutput").ap()

    with tile.TileContext(nc) as tc:
        with tc.tile_pool(name="wp", bufs=1) as wp, \
             tc.tile_pool(name="xkp", bufs=2) as xkp, \
             tc.tile_pool(name="gp", bufs=2) as gp, \
             tc.tile_pool(name="g1", bufs=1) as g1, \
             tc.tile_pool(name="cpool", bufs=1) as cpool, \
             tc.tile_pool(name="psum", bufs=8, space="PSUM") as psum, \
             tc.tile_pool(name="dstage", bufs=2, space="DRAM") as dstage, \
             tc.tile_pool(name="dfull", bufs=8, space="DRAM") as dfull:

            # gathered h1 blocks persist through layer 2
            h1f8 = [dfull.tile([D, TBLK], fp8, tag="h1f8", name=f"h1f8_{t}")
                    for t in range(NBLK)] if use8 else None
            h1fb = [dfull.tile([D, TBLK], bf16, tag="h1fb", name=f"h1fb_{t}")
                    for t in range(NBLK)] if use16 else None

            for l in range(2):
                # weights/bias DMAs ride the (idle) gpsimd queue so the
                # first token block's x DMAs on the sync queue aren't
                # serialized behind ~3MB of weight traffic
                w8k = []
                if use8:
                    for kk in range(NKP):
                        wk = wp.tile([128, 2, C8], fp8, tag=f"w8_{kk}",
                                     name=f"w8_{l}_{kk}")
                        nc.gpsimd.dma_start(
                            wk[:],
                            w8_d[l][kk * 128:(kk + 1) * 128, :]
                            .rearrange("p (s c) -> p s c", s=2))
                        w8k.append(wk)
                w16k = []
                if use16:
                    for kk in range(NKP):
                        wk = wp.tile([128, 2, C16], bf16, tag=f"w16_{kk}",
                                     name=f"w16_{l}_{kk}")
                        nc.gpsimd.dma_start(
                            wk[:],
                            w16_d[l][kk * 256:(kk + 1) * 256, :]
                            .rearrange("(s p) c -> p s c", s=2))
                        w16k.append(wk)
                ba = cpool.tile([128, 16], f32, tag=f"ba{l}", name=f"ba{l}")
                nc.gpsimd.dma_start(ba[:], ba_d[l][:])
                bc = cpool.tile([128, 4], f32, tag=f"bc{l}", name=f"bc{l}")
                nc.gpsimd.dma_start(bc[:], bc_d[l][:])
                cp = cpool.tile([128, 4], f32, tag=f"cp{l}", name=f"cp{l}")
                nc.gpsimd.dma_start(cp[:], cp_d[l][:])

                carry = [None] * NCHUNK
                # Token blocks are processed in PAIRS: the ACT-engine phases
                # (sigmoid table vs ln/exp table) are emitted grouped across
                # both blocks of a pair, so lower_act inserts only 2 table
                # loads per pair instead of per block (1283ns each).
                pend = []
                for t in range(NBLK):
                    # ---- input tiles for this token block ----
                    x8k = []
                    if use8:
                        for kk in range(NKP):
                            xt = xkp.tile([128, 2, TBLK], fp8, tag=f"x8_{kk}",
                                          name=f"x8_{l}_{t}_{kk}")
                            if l == 0:
                                src = x8_d[kk * 256:(kk + 1) * 256,
                                           t * TBLK:(t + 1) * TBLK]
                            else:
                                src = h1f8[t][kk * 256:(kk + 1) * 256, :]
                            nc.sync.dma_start(
                                xt[:], src.rearrange("(s p) n -> p s n", s=2))
                            x8k.append(xt)
                    xbk = []
                    if use16:
                        for kk in range(NKP):
                            xt = xkp.tile([128, 2, TBLK], bf16, tag=f"xb_{kk}",
                                          name=f"xb_{l}_{t}_{kk}")
                            if l == 0:
                                src = xb_d[kk * 256:(kk + 1) * 256,
                                           t * TBLK:(t + 1) * TBLK]
                            else:
                                src = h1fb[t][kk * 256:(kk + 1) * 256, :]
                            nc.sync.dma_start(
                                xt[:], src.rearrange("(s p) n -> p s n", s=2))
                            xbk.append(xt)

                    if l == 0:
                        if use8:
                            h8own = dstage.tile([HALF, TBLK], fp8, tag="h8own",
                                                name=f"h8own{t}")
                        if use16:
                            hbown = dstage.tile([HALF, TBLK], bf16, tag="hbown",
                                                name=f"hbown{t}")

                    # ---- matmuls into PSUM, per (j, quarter) ----
                    ps = {}
                    for j in range(NCHUNK):
                        for q in QNAMES:
                            p = psum.tile([128, TBLK], f32, tag="ps",
                                          name=f"ps_{q}{l}_{t}_{j}")
                            if q in FP8_GATES:
                                off = q8list.index(q) * HALF + j * 128
                                for kk in range(NKP):
                                    nc.tensor.matmul(
                                        p[:], w8k[kk][:, :, off:off + 128],
                                        x8k[kk][:],
                                        start=(kk == 0), stop=(kk == NKP - 1),
                                        perf_mode=DR)
                            else:
                                off = qblist.index(q) * HALF + j * 128
                                for k in range(NKT):
                                    kk, s = k // 2, k % 2
                                    nc.tensor.matmul(
                                        p[:],
                                        w16k[kk][:, s, off:off + 128],
                                        xbk[kk][:, s, :],
                                        start=(k == 0), stop=(k == NKT - 1))
                            ps[(j, q)] = p

                    # ---- wide (j-pair) elementwise tiles ----
                    # gp (bufs=2): tiles crossing engines between iterations;
                    # g1 (bufs=1): tiles produced+consumed in DVE program order
                    def wtile(pool, nm, dt):
                        return [pool.tile([128, 2, TBLK], dt, tag=f"{nm}{p}",
                                          name=f"{nm}{p}_{l}_{t}")
                                for p in range(2)]

                    sf = wtile(gp, "sf", ew("sf"))
                    si = wtile(gp, "si", ew("sf"))
                    sg = wtile(gp, "sg", ew("sg"))
                    so = wtile(gp, "so", ew("so"))
                    cp5 = wtile(gp, "cp5", ew("sg"))
                    ssum = wtile(gp, "ssum", ew("sf"))
                    lns = wtile(gp, "lns", f32)
                    rr = wtile(gp, "rr", ew("sf"))
                    aa = wtile(g1, "aa", ew("a"))
                    ipr = wtile(g1, "ipr", ew("a"))
                    gg = wtile(gp, "gg", ew("g"))
                    bt = wtile(g1, "bt", ew("bt"))
                    cc = wtile(gp, "cc", ew("c"))
                    hh = wtile(gp, "hh", bf16)
                    h8 = wtile(gp, "h8", fp8) if (l == 0 and use8) else None

                    def scl(q):
                        return 1.0 / SC if q in FP8_GATES else 1.0

                    def bcol(qi, j):
                        return ba[:, qi * NCHUNK + j:qi * NCHUNK + j + 1]

                    # ACT sigmoid phase (one table): per j
                    for j in range(NCHUNK):
                        p, jj = j // 2, j % 2
                        nc.scalar.activation(sf[p][:, jj, :], ps[(j, "f")][:],
                                             AF.Sigmoid, bias=bcol(1, j),
                                             scale=scl("f"))
                        nc.scalar.activation(si[p][:, jj, :], ps[(j, "i")][:],
                                             AF.Sigmoid, bias=bcol(0, j),
                                             scale=scl("i"))
                        nc.scalar.activation(sg[p][:, jj, :], ps[(j, "cell")][:],
                                             AF.Sigmoid, bias=bcol(3, j),
                                             scale=scl("cell"))
                        nc.scalar.activation(so[p][:, jj, :], ps[(j, "o")][:],
                                             AF.Sigmoid, bias=bcol(2, j),
                                             scale=scl("o"))
                        # cp5 = cell/SC + bc + 0.5 on DVE (frees the psum bank)
                        if "cell" in FP8_GATES:
                            nc.vector.tensor_scalar(
                                cp5[p][:, jj, :], ps[(j, "cell")][:],
                                1.0 / SC, bc[:, j:j + 1], ALU.mult, ALU.add)
                        else:
                            nc.vector.tensor_scalar(
                                cp5[p][:, jj, :], ps[(j, "cell")][:],
                                bc[:, j:j + 1], None, ALU.add)

                    # DVE: ssum per pair
                    for p in range(2):
                        nc.vector.tensor_tensor(ssum[p][:], sf[p][:], si[p][:],
                                                ALU.add)

                    pend.append(dict(
                        t=t, sf=sf, si=si, sg=sg, so=so, cp5=cp5, ssum=ssum,
                        lns=lns, rr=rr, aa=aa, ipr=ipr, gg=gg, bt=bt, cc=cc,
                        hh=hh, h8=h8,
                        h8own=h8own if (l == 0 and use8) else None,
                        hbown=hbown if (l == 0 and use16) else None))
                    # batch the ACT table phases across block pairs, EXCEPT
                    # the last two blocks of layer 2: pairing there puts a
                    # full extra block of elementwise latency on the kernel
                    # tail after the final matmul (2 extra table loads are
                    # cheaper than ~15us of un-overlapped tail)
                    unpaired = (l == 1 and t >= NBLK - 2)
                    if len(pend) < 2 and not unpaired:
                        continue

                    # ACT ln then exp phases for the whole pair (ln and exp
                    # live in different act tables in this compiler build, so
                    # group all Lns together then all Exps)
                    for st in pend:
                        for p in range(2):
                            nc.scalar.activation(st["lns"][p][:],
                                                 st["ssum"][p][:], AF.Ln)
                    for st in pend:
                        for p in range(2):
                            nc.scalar.activation(st["rr"][p][:],
                                                 st["lns"][p][:], AF.Exp,
                                                 scale=-1.0)
                    for st in pend:
                        tt = st["t"]
                        # DVE: a, ipr, g, bt per pair
                        for p in range(2):
                            nc.vector.tensor_tensor(
                                st["aa"][p][:], st["sf"][p][:],
                                st["rr"][p][:], ALU.mult)
                            nc.vector.tensor_scalar(
                                st["ipr"][p][:], st["aa"][p][:], -1.0, 1.0,
                                ALU.mult, ALU.add)
                            nc.vector.tensor_tensor(
                                st["gg"][p][:], st["cp5"][p][:],
                                st["sg"][p][:], ALU.max)
                            nc.vector.tensor_tensor(
                                st["bt"][p][:], st["ipr"][p][:],
                                st["gg"][p][:], ALU.mult)
                        # DVE: scans per j (recurrence along tokens)
                        for j in range(NCHUNK):
                            p, jj = j // 2, j % 2
                            init = cp[:, j:j + 1] if tt == 0 else carry[j]
                            nc.vector.tensor_tensor_scan(
                                st["cc"][p][:, jj, :], st["aa"][p][:, jj, :],
                                st["bt"][p][:, jj, :],
                                init, ALU.mult, ALU.add)
                            carry[j] = st["cc"][p][:, jj, TBLK - 1:TBLK]
                        # DVE: h (+ fp8 copy for layer-2 rhs)
                        for p in range(2):
                            nc.vector.tensor_tensor(
                                st["hh"][p][:], st["so"][p][:],
                                st["cc"][p][:], ALU.mult)
                            if st["h8"] is not None:
                                nc.vector.tensor_tensor(
                                    st["h8"][p][:], st["so"][p][:],
                                    st["cc"][p][:], ALU.mult)

                        # ---- outputs ----
                        if l == 0:
                            for p in range(2):
                                if use16:
                                    nc.sync.dma_start(
                                        st["hbown"][p * 256:(p + 1) * 256, :]
                                        .rearrange("(s q) n -> q s n", s=2),
                                        st["hh"][p][:])
                                if use8:
                                    nc.sync.dma_start(
                                        st["h8own"][p * 256:(p + 1) * 256, :]
                                        .rearrange("(s q) n -> q s n", s=2),
                                        st["h8"][p][:])
                            if use16:
                                nc.gpsimd.collective_compute(
                                    "AllGather", ALU.bypass,
                                    replica_groups=[[0, 1], [2, 3],
                                                    [4, 5], [6, 7]],
                                    ins=[st["hbown"].opt()],
                                    outs=[h1fb[tt].opt()],
                                )
                            if use8:
                                nc.gpsimd.collective_compute(
                                    "AllGather", ALU.bypass,
                                    replica_groups=[[0, 1], [2, 3],
                                                    [4, 5], [6, 7]],
                                    ins=[st["h8own"].opt()],
                                    outs=[h1f8[tt].opt()],
                                )
                        else:
                            for p in range(2):
                                nc.sync.dma_start(
                                    h2t_d[p * 256:(p + 1) * 256,
                                          tt * TBLK:(tt + 1) * TBLK]
                                    .rearrange("(s q) n -> q s n", s=2),
                                    st["hh"][p][:])
                    pend = []

    _split_multi_waits(nc)
    return nc


def _shard_inputs(x, W0, b0, W1, b1, c0_prev, c1_prev):
    import ml_dtypes
    f8 = ml_dtypes.float8_e4m3
    bf = ml_dtypes.bfloat16

    q8list = [q for q in QNAMES if q in FP8_GATES]
    qblist = [q for q in QNAMES if q not in FP8_GATES]
    qidx = {q: i for i, q in enumerate(QNAMES)}
    C8 = len(q8list) * HALF
    C16 = len(qblist) * HALF

    x = np.asarray(x, dtype=np.float32)
    xT = [np.ascontiguousarray(x[b].T) for b in range(B)]
    x8T = [t.astype(f8) for t in xT] if C8 else None
    xbT = [t.astype(bf) for t in xT] if C16 else None

    per_layer = []
    for (W, bb) in ((W0, b0), (W1, b1)):
        W = np.asarray(W, dtype=np.float32)
        bb = np.asarray(bb, dtype=np.float32)
        halves = []
        for h in range(2):
            if C8:
                w8 = np.empty((NKP, 128, 2, C8), dtype=f8)
                for kk in range(NKP):
                    for s in range(2):
                        rows = np.concatenate(
                            [qidx[q] * D + h * HALF + np.arange(HALF)
                             for q in q8list])
                        blk = W[rows, (2 * kk + s) * 128:(2 * kk + s + 1) * 128]
                        w8[kk, :, s, :] = (blk.T * np.float32(SC)).astype(f8)
                w8 = np.ascontiguousarray(w8.reshape(NKP * 128, 2 * C8))
            else:
                w8 = None
            if C16:
                w16 = np.empty((NKT, 128, C16), dtype=bf)
                for k in range(NKT):
                    rows = np.concatenate(
                        [qidx[q] * D + h * HALF + np.arange(HALF)
                         for q in qblist])
                    blk = W[rows, k * 128:(k + 1) * 128]
                    w16[k] = blk.T.astype(bf)
                w16 = np.ascontiguousarray(w16.reshape(NKT * 128, C16))
            else:
                w16 = None
            rows_all = np.concatenate(
                [q * D + h * HALF + np.arange(HALF) for q in range(4)])
            ba = np.ascontiguousarray(bb[rows_all].reshape(16, 128).T)
            bc = np.ascontiguousarray(ba[:, 12:16] + np.float32(0.5))
            halves.append((w8, w16, ba, bc))
        per_layer.append(halves)

    cps = []
    for cprev in (c0_prev, c1_prev):
        cprev = np.asarray(cprev, dtype=np.float32)
        halves = []
        for b in range(B):
            row = []
            for h in range(2):
                seg = cprev[b, 0, h * HALF:(h + 1) * HALF]
                row.append(np.ascontiguousarray(seg.reshape(4, 128).T))
            halves.append(row)
        cps.append(halves)

    in_maps = []
    for k in range(NCORES):
        b, h = k // 2, k % 2
        m = {}
        if C8:
            m["x8T"] = x8T[b]
        if C16:
            m["xbT"] = xbT[b]
        for l in range(2):
            w8, w16, ba, bc = per_layer[l][h]
            if C8:
                m[f"w8_{l}"] = w8
            if C16:
                m[f"w16_{l}"] = w16
            m[f"b{l}a"] = ba
            m[f"b{l}c"] = bc
            m[f"cp{l}"] = cps[l][b][h]
        in_maps.append(m)
    return in_maps


def _get_nc():
    if "nc" not in _CACHE:
        _CACHE["nc"] = _build_nc()
    return _CACHE["nc"]


def kernel(x, W0, b0, W1, b1, c0_prev, c1_prev):
    from concourse.bass_utils import run_bass_kernel_spmd

    nc = _get_nc()
    in_maps = _shard_inputs(x, W0, b0, W1, b1, c0_prev, c1_prev)
    res = run_bass_kernel_spmd(nc, in_maps, list(range(NCORES)))
    out = np.empty((B, S, D), dtype=np.float32)
    for k in range(NCORES):
        b, h = k // 2, k % 2
        out[b, :, h * HALF:(h + 1) * HALF] = \
            res.results[k]["h2t"].astype(np.float32).T
    return out


# revision 21
# speedup vs baseline: 1.0714x; 1.0714x over previous
"""minLSTM (2-layer, B=4, S=4096, D=1024) on 8 Trainium2 NeuronCores.

Sharding: core k -> (batch b = k//2, channel half h = k%2).
Each core computes all 4096 timesteps for its batch and its 512 channels.

Key optimizations over the f32r baseline (629us):
  - per-gate matmul dtypes: i/f gates via fp8e4m3 DoubleRow matmuls (2 k-tiles
    per instruction = 2x PE throughput; the i'/f' normalization suppresses
    their quantization noise), o/cell gates in bf16 (error-sensitive paths).
    Weights scaled by 64 into fp8 range; descale rides the ACT `scale`.
  - the f'=sig(f)/(sig(f)+sig(i)) division moved off the DVE: the 3352ns
    DVE RECIPROCAL becomes r = Exp(-Ln(ssum)) on the ACT engine (ln and exp
    share one activation table; sigmoid phases and ln/exp phases are batched
    per token block so only 2 table loads per block are inserted).
  - elementwise ops run 1024-wide on j-chunk pairs (fixed op overhead halved),
    in bf16 where the error budget allows (2x DVE throughput).
  - recurrence via DVE tensor_tensor_scan in linear space, f32 internal state.
Between layers, channel-half pairs exchange h1 via pairwise AllGather in both
bf16 (for bf16-gate rhs) and fp8 (for DoubleRow rhs), overlapped with compute.

Self-contained: hardcodes shapes; only imports the system concourse repo.
"""
import os
import sys

if '/opt/trn_rl_repo' not in sys.path:
    sys.path.insert(0, '/opt/trn_rl_repo')

import numpy as np

B, S, D = 4, 4096, 1024
NCORES = 8
HALF = D // 2           # channels per core: 512
NCHUNK = HALF // 128    # 4 partition chunks of 128 channels
NKT = D // 128          # 8 contraction k-tiles
NKP = NKT // 2          # 4 contraction k-pairs (fp8 DoubleRow)
TBLK = 512              # token block
NBLK = S // TBLK        # 8 token blocks
GCH = 4 * HALF          # gate channels per core: 2048
SC = 64.0               # fp8 weight scale

QNAMES = ("i", "f", "o", "cell")
# which gate quarters run through fp8 DoubleRow matmuls
FP8_GATES = set(os.environ.get("MINLSTM_FP8", "if"))         # subset of {i,f,o,c}
FP8_GATES = {q for q in QNAMES if q[0] in FP8_GATES}
# which elementwise values are computed/stored in bf16
EW_BF16 = set(os.environ.get("MINLSTM_BF16", "h,so,sg,g,bt,a,sf").split(","))

_CACHE = {}


def _split_multi_waits(nc):
    """This walrus build rejects >1 sync wait per instruction. Hoist extra
    waits onto same-engine NoOps inserted just before; engine-queue program
    order makes this semantically identical."""
    from concourse import mybir
    n = 0
    for fn in nc.m.functions:
        for blk in fn.blocks:
            insts = list(blk.instructions)
            new = []
            changed = False
            for inst in insts:
                si = inst.sync_info
                ow = list(si.on_wait) if si is not None and si.on_wait else []
                if len(ow) > 1:
                    changed = True
                    for w in ow[:-1]:
                        n += 1
                        nop = mybir.InstNoOp(name=f"I-wsplit-{n}", ins=[], outs=[])
                        nop.engine = inst.engine
                        nop.sync_info = mybir.SyncInfo(on_wait=[w], on_update=[])
                        new.append(nop)
                    si.on_wait = [ow[-1]]
                new.append(inst)
            if changed:
                blk.instructions = new
    return n


def _build_nc():
    import concourse.bass as bass
    import concourse.mybir as mybir
    import concourse.tile as tile

    f32 = mybir.dt.float32
    bf16 = mybir.dt.bfloat16
    fp8 = mybir.dt.float8e4
    AF = mybir.ActivationFunctionType
    ALU = mybir.AluOpType
    DR = mybir.MatmulPerfMode.DoubleRow

    def ew(name):
        return bf16 if name in EW_BF16 else f32

    q8list = [q for q in QNAMES if q in FP8_GATES]
    qblist = [q for q in QNAMES if q not in FP8_GATES]
    C8 = len(q8list) * HALF      # fp8 weight columns per core
    C16 = len(qblist) * HALF     # bf16 weight columns per core
    use8 = C8 > 0
    use16 = C16 > 0

    nc = bass.Bass("TRN2", target_bir_lowering=False, debug=False,
                   num_devices=NCORES)

    x8_d = nc.dram_tensor("x8T", [D, S], fp8, kind="ExternalInput").ap() \
        if use8 else None
    xb_d = nc.dram_tensor("xbT", [D, S], bf16, kind="ExternalInput").ap() \
        if use16 else None
    w8_d = [nc.dram_tensor(f"w8_{l}", [NKP * 128, 2 * C8], fp8,
                           kind="ExternalInput").ap() if use8 else None
            for l in range(2)]
    w16_d = [nc.dram_tensor(f"w16_{l}", [NKT * 128, C16], bf16,
                            kind="ExternalInput").ap() if use16 else None
             for l in range(2)]
    ba_d = [nc.dram_tensor(f"b{l}a", [128, 16], f32, kind="ExternalInput").ap()
            for l in range(2)]
    bc_d = [nc.dram_tensor(f"b{l}c", [128, 4], f32, kind="ExternalInput").ap()
            for l in range(2)]
    cp_d = [nc.dram_tensor(f"cp{l}", [128, 4], f32, kind="ExternalInput").ap()
            for l in range(2)]
    h2t_d = nc.dram_tensor("h2t", [HALF, S], bf16, kind="ExternalOutput").ap()

    with tile.TileContext(nc) as tc:
        with tc.tile_pool(name="wp", bufs=1) as wp, \
             tc.tile_pool(name="xkp", bufs=2) as xkp, \
             tc.tile_pool(name="gp", bufs=2) as gp, \
             tc.tile_pool(name="g1", bufs=1) as g1, \
             tc.tile_pool(name="cpool", bufs=1) as cpool, \
             tc.tile_pool(name="psum", bufs=8, space="PSUM") as psum, \
             tc.tile_pool(name="dstage", bufs=2, space="DRAM") as dstage, \
             tc.tile_pool(name="dfull", bufs=8, space="DRAM") as dfull:

            # gathered h1 blocks persist through layer 2
            h1f8 = [dfull.tile([D, TBLK], fp8, tag="h1f8", name=f"h1f8_{t}")
                    for t in range(NBLK)] if use8 else None
            h1fb = [dfull.tile([D, TBLK], bf16, tag="h1fb", name=f"h1fb_{t}")
                    for t in range(NBLK)] if use16 else None

            for l in range(2):
                # fp8 weights load first: the i/f-gate matmuls of the first
                # token block need only these, so the PE starts ~6us earlier;
                # the bf16 weights are deferred until after t0's x tiles.
                w8k = []
                if use8:
                    for kk in range(NKP):
                        wk = wp.tile([128, 2, C8], fp8, tag=f"w8_{kk}",
                                     name=f"w8_{l}_{kk}")
                        nc.sync.dma_start(
                            wk[:],
                            w8_d[l][kk * 128:(kk + 1) * 128, :]
                            .rearrange("p (s c) -> p s c", s=2))
                        w8k.append(wk)
                ba = cpool.tile([128, 16], f32, tag=f"ba{l}", name=f"ba{l}")
                nc.sync.dma_start(ba[:], ba_d[l][:])
                bc = cpool.tile([128, 4], f32, tag=f"bc{l}", name=f"bc{l}")
                nc.sync.dma_start(bc[:], bc_d[l][:])
                cp = cpool.tile([128, 4], f32, tag=f"cp{l}", name=f"cp{l}")
                nc.sync.dma_start(cp[:], cp_d[l][:])

                w16k = []
                carry = [None] * NCHUNK
                # Token blocks are processed in PAIRS: the ACT-engine phases
                # (sigmoid table vs ln/exp table) are emitted grouped across
                # both blocks of a pair, so lower_act inserts only 2 table
                # loads per pair instead of per block (1283ns each).
                pend = []
                for t in range(NBLK):
                    # ---- input tiles for this token block ----
                    x8k = []
                    if use8:
                        for kk in range(NKP):
                            xt = xkp.tile([128, 2, TBLK], fp8, tag=f"x8_{kk}",
                                          name=f"x8_{l}_{t}_{kk}")
                            if l == 0:
                                src = x8_d[kk * 256:(kk + 1) * 256,
                                           t * TBLK:(t + 1) * TBLK]
                            else:
                                src = h1f8[t][kk * 256:(kk + 1) * 256, :]
                            nc.sync.dma_start(
                                xt[:], src.rearrange("(s p) n -> p s n", s=2))
                            x8k.append(xt)
                    xbk = []
                    if use16:
                        for kk in range(NKP):
                            xt = xkp.tile([128, 2, TBLK], bf16, tag=f"xb_{kk}",
                                          name=f"xb_{l}_{t}_{kk}")
                            if l == 0:
                                src = xb_d[kk * 256:(kk + 1) * 256,
                                           t * TBLK:(t + 1) * TBLK]
                            else:
                                src = h1fb[t][kk * 256:(kk + 1) * 256, :]
                            nc.sync.dma_start(
                                xt[:], src.rearrange("(s p) n -> p s n", s=2))
                            xbk.append(xt)

                    if t == 0 and use16:
                        # bf16 weights deferred behind t0's x tiles
                        w16k.clear()
                        for kk in range(NKP):
                            wk = wp.tile([128, 2, C16], bf16, tag=f"w16_{kk}",
                                         name=f"w16_{l}_{kk}")
                            nc.sync.dma_start(
                                wk[:],
                                w16_d[l][kk * 256:(kk + 1) * 256, :]
                                .rearrange("(s p) c -> p s c", s=2))
                            w16k.append(wk)

                    if l == 0:
                        if use8:
                            h8own = dstage.tile([HALF, TBLK], fp8, tag="h8own",
                                                name=f"h8own{t}")
                        if use16:
                            hbown = dstage.tile([HALF, TBLK], bf16, tag="hbown",
                                                name=f"hbown{t}")

                    # ---- matmuls into PSUM, per (j, quarter) ----
                    ps = {}
                    for j in range(NCHUNK):
                        for q in QNAMES:
                            p = psum.tile([128, TBLK], f32, tag="ps",
                                          name=f"ps_{q}{l}_{t}_{j}")
                            if q in FP8_GATES:
                                off = q8list.index(q) * HALF + j * 128
                                for kk in range(NKP):
                                    nc.tensor.matmul(
                                        p[:], w8k[kk][:, :, off:off + 128],
                                        x8k[kk][:],
                                        start=(kk == 0), stop=(kk == NKP - 1),
                                        perf_mode=DR)
                            else:
                                off = qblist.index(q) * HALF + j * 128
                                for k in range(NKT):
                                    kk, s = k // 2, k % 2
                                    nc.tensor.matmul(
                                        p[:],
                                        w16k[kk][:, s, off:off + 128],
                                        xbk[kk][:, s, :],
                                        start=(k == 0), stop=(k == NKT - 1))
                            ps[(j, q)] = p

                    # ---- wide (j-pair) elementwise tiles ----
                    # gp (bufs=2): tiles crossing engines between iterations;
                    # g1 (bufs=1): tiles produced+consumed in DVE program order
                    def wtile(pool, nm, dt):
                        return [pool.tile([128, 2, TBLK], dt, tag=f"{nm}{p}",
                                          name=f"{nm}{p}_{l}_{t}")
                                for p in range(2)]

                    sf = wtile(gp, "sf", ew("sf"))
                    si = wtile(gp, "si", ew("sf"))
                    sg = wtile(gp, "sg", ew("sg"))
                    so = wtile(gp, "so", ew("so"))
                    cp5 = wtile(gp, "cp5", ew("sg"))
                    ssum = wtile(gp, "ssum", ew("sf"))
                    lns = wtile(gp, "lns", f32)
                    rr = wtile(gp, "rr", ew("sf"))
                    aa = wtile(g1, "aa", ew("a"))
                    ipr = wtile(g1, "ipr", ew("a"))
                    gg = wtile(gp, "gg", ew("g"))
                    bt = wtile(g1, "bt", ew("bt"))
                    cc = wtile(gp, "cc", ew("c"))
                    hh = wtile(gp, "hh", bf16)
                    h8 = wtile(gp, "h8", fp8) if (l == 0 and use8) else None

                    def scl(q):
                        return 1.0 / SC if q in FP8_GATES else 1.0

                    def bcol(qi, j):
                        return ba[:, qi * NCHUNK + j:qi * NCHUNK + j + 1]

                    # ACT sigmoid phase (one table): per j
                    for j in range(NCHUNK):
                        p, jj = j // 2, j % 2
                        nc.scalar.activation(sf[p][:, jj, :], ps[(j, "f")][:],
                                             AF.Sigmoid, bias=bcol(1, j),
                                             scale=scl("f"))
                        nc.scalar.activation(si[p][:, jj, :], ps[(j, "i")][:],
                                             AF.Sigmoid, bias=bcol(0, j),
                                             scale=scl("i"))
                        nc.scalar.activation(sg[p][:, jj, :], ps[(j, "cell")][:],
                                             AF.Sigmoid, bias=bcol(3, j),
                                             scale=scl("cell"))
                        nc.scalar.activation(so[p][:, jj, :], ps[(j, "o")][:],
                                             AF.Sigmoid, bias=bcol(2, j),
                                             scale=scl("o"))
                        # cp5 = cell/SC + bc + 0.5 on DVE (frees the psum bank)
                        if "cell" in FP8_GATES:
                            nc.vector.tensor_scalar(
                                cp5[p][:, jj, :], ps[(j, "cell")][:],
                                1.0 / SC, bc[:, j:j + 1], ALU.mult, ALU.add)
                        else:
                            nc.vector.tensor_scalar(
                                cp5[p][:, jj, :], ps[(j, "cell")][:],
                                bc[:, j:j + 1], None, ALU.add)

                    # DVE: ssum per pair
                    for p in range(2):
                        nc.vector.tensor_tensor(ssum[p][:], sf[p][:], si[p][:],
                                                ALU.add)

                    pend.append(dict(
                        t=t, sf=sf, si=si, sg=sg, so=so, cp5=cp5, ssum=ssum,
                        lns=lns, rr=rr, aa=aa, ipr=ipr, gg=gg, bt=bt, cc=cc,
                        hh=hh, h8=h8,
                        h8own=h8own if (l == 0 and use8) else None,
                        hbown=hbown if (l == 0 and use16) else None))
                    # batch the ACT table phases across block pairs, EXCEPT
                    # the last two blocks of layer 2: pairing there puts a
                    # full extra block of elementwise latency on the kernel
                    # tail after the final matmul (2 extra table loads are
                    # cheaper than ~15us of un-overlapped tail)
                    unpaired = (l == 1 and t >= NBLK - 2)
                    if len(pend) < 2 and not unpaired:
                        continue

                    # ACT ln then exp phases for the whole pair (ln and exp
                    # live in different act tables in this compiler build, so
                    # group all Lns together then all Exps)
                    for st in pend:
                        for p in range(2):
                            nc.scalar.activation(st["lns"][p][:],
                                                 st["ssum"][p][:], AF.Ln)
                    for st in pend:
                        for p in range(2):
                            nc.scalar.activation(st["rr"][p][:],
                                                 st["lns"][p][:], AF.Exp,
                                                 scale=-1.0)
                    for st in pend:
                        tt = st["t"]
                        # DVE: a, ipr, g, bt per pair
                        for p in range(2):
                            nc.vector.tensor_tensor(
                                st["aa"][p][:], st["sf"][p][:],
                                st["rr"][p][:], ALU.mult)
                            nc.vector.tensor_scalar(
                                st["ipr"][p][:], st["aa"][p][:], -1.0, 1.0,
                                ALU.mult, ALU.add)
                            nc.vector.tensor_tensor(
                                st["gg"][p][:], st["cp5"][p][:],
                                st["sg"][p][:], ALU.max)
                            nc.vector.tensor_tensor(
                                st["bt"][p][:], st["ipr"][p][:],
                                st["gg"][p][:], ALU.mult)
                        # DVE: scans per j (recurrence along tokens)
                        for j in range(NCHUNK):
                            p, jj = j // 2, j % 2
                            init = cp[:, j:j + 1] if tt == 0 else carry[j]
                            nc.vector.tensor_tensor_scan(
                                st["cc"][p][:, jj, :], st["aa"][p][:, jj, :],
                                st["bt"][p][:, jj, :],
                                init, ALU.mult, ALU.add)
                            carry[j] = st["cc"][p][:, jj, TBLK - 1:TBLK]
                        # DVE: h (+ fp8 copy for layer-2 rhs)
                        for p in range(2):
                            nc.vector.tensor_tensor(
                                st["hh"][p][:], st["so"][p][:],
                                st["cc"][p][:], ALU.mult)
                            if st["h8"] is not None:
                                nc.vector.tensor_tensor(
                                    st["h8"][p][:], st["so"][p][:],
                                    st["cc"][p][:], ALU.mult)

                        # ---- outputs ----
                        if l == 0:
                            for p in range(2):
                                if use16:
                                    nc.sync.dma_start(
                                        st["hbown"][p * 256:(p + 1) * 256, :]
                                        .rearrange("(s q) n -> q s n", s=2),
                                        st["hh"][p][:])
                                if use8:
                                    nc.sync.dma_start(
                                        st["h8own"][p * 256:(p + 1) * 256, :]
                                        .rearrange("(s q) n -> q s n", s=2),
                                        st["h8"][p][:])
                            if use16:
                                nc.gpsimd.collective_compute(
                                    "AllGather", ALU.bypass,
                                    replica_groups=[[0, 1], [2, 3],
                                                    [4, 5], [6, 7]],
                                    ins=[st["hbown"].opt()],
                                    outs=[h1fb[tt].opt()],
                                )
                            if use8:
                                nc.gpsimd.collective_compute(
                                    "AllGather", ALU.bypass,
                                    replica_groups=[[0, 1], [2, 3],
                                                    [4, 5], [6, 7]],
                                    ins=[st["h8own"].opt()],
                                    outs=[h1f8[tt].opt()],
                                )
                        else:
                            for p in range(2):
                                nc.sync.dma_start(
                                    h2t_d[p * 256:(p + 1) * 256,
                                          tt * TBLK:(tt + 1) * TBLK]
                                    .rearrange("(s q) n -> q s n", s=2),
                                    st["hh"][p][:])
                    pend = []

    _split_multi_waits(nc)
    return nc


def _shard_inputs(x, W0, b0, W1, b1, c0_prev, c1_prev):
    import ml_dtypes
    f8 = ml_dtypes.float8_e4m3
    bf = ml_dtypes.bfloat16

    q8list = [q for q in QNAMES if q in FP8_GATES]
    qblist = [q for q in QNAMES if q not in FP8_GATES]
    qidx = {q: i for i, q in enumerate(QNAMES)}
    C8 = len(q8list) * HALF
    C16 = len(qblist) * HALF

    x = np.asarray(x, dtype=np.float32)
    xT = [np.ascontiguousarray(x[b].T) for b in range(B)]
    x8T = [t.astype(f8) for t in xT] if C8 else None
    xbT = [t.astype(bf) for t in xT] if C16 else None

    per_layer = []
    for (W, bb) in ((W0, b0), (W1, b1)):
        W = np.asarray(W, dtype=np.float32)
        bb = np.asarray(bb, dtype=np.float32)
        halves = []
        for h in range(2):
            if C8:
                w8 = np.empty((NKP, 128, 2, C8), dtype=f8)
                for kk in range(NKP):
                    for s in range(2):
                        rows = np.concatenate(
                            [qidx[q] * D + h * HALF + np.arange(HALF)
                             for q in q8list])
                        blk = W[rows, (2 * kk + s) * 128:(2 * kk + s + 1) * 128]
                        w8[kk, :, s, :] = (blk.T * np.float32(SC)).astype(f8)
                w8 = np.ascontiguousarray(w8.reshape(NKP * 128, 2 * C8))
            else:
                w8 = None
            if C16:
                w16 = np.empty((NKT, 128, C16), dtype=bf)
                for k in range(NKT):
                    rows = np.concatenate(
                        [qidx[q] * D + h * HALF + np.arange(HALF)
                         for q in qblist])
                    blk = W[rows, k * 128:(k + 1) * 128]
                    w16[k] = blk.T.astype(bf)
                w16 = np.ascontiguousarray(w16.reshape(NKT * 128, C16))
            else:
                w16 = None
            rows_all = np.concatenate(
                [q * D + h * HALF + np.arange(HALF) for q in range(4)])
            ba = np.ascontiguousarray(bb[rows_all].reshape(16, 128).T)
            bc = np.ascontiguousarray(ba[:, 12:16] + np.float32(0.5))
            halves.append((w8, w16, ba, bc))
        per_layer.append(halves)

    cps = []
    for cprev in (c0_prev, c1_prev):
        cprev = np.asarray(cprev, dtype=np.float32)
        halves = []
        for b in range(B):
            row = []
            for h in range(2):
                seg = cprev[b, 0, h * HALF:(h + 1) * HALF]
                row.append(np.ascontiguousarray(seg.reshape(4, 128).T))
            halves.append(row)
        cps.append(halves)

    in_maps = []
    for k in range(NCORES):
        b, h = k // 2, k % 2
        m = {}
        if C8:
            m["x8T"] = x8T[b]
        if C16:
            m["xbT"] = xbT[b]
        for l in range(2):
            w8, w16, ba, bc = per_layer[l][h]
            if C8:
                m[f"w8_{l}"] = w8
            if C16:
                m[f"w16_{l}"] = w16
            m[f"b{l}a"] = ba
            m[f"b{l}c"] = bc
            m[f"cp{l}"] = cps[l][b][h]
        in_maps.append(m)
    return in_maps


def _get_nc():
    if "nc" not in _CACHE:
        _CACHE["nc"] = _build_nc()
    return _CACHE["nc"]


def kernel(x, W0, b0, W1, b1, c0_prev, c1_prev):
    from concourse.bass_utils import run_bass_kernel_spmd

    nc = _get_nc()
    in_maps = _shard_inputs(x, W0, b0, W1, b1, c0_prev, c1_prev)
    res = run_bass_kernel_spmd(nc, in_maps, list(range(NCORES)))
    out = np.empty((B, S, D), dtype=np.float32)
    for k in range(NCORES):
        b, h = k // 2, k % 2
        out[b, :, h * HALF:(h + 1) * HALF] = \
            res.results[k]["h2t"].astype(np.float32).T
    return out


# revision 26
# speedup vs baseline: 1.0751x; 1.0035x over previous
"""minLSTM (2-layer, B=4, S=4096, D=1024) on 8 Trainium2 NeuronCores.

Sharding: core k -> (batch b = k//2, channel half h = k%2).
Each core computes all 4096 timesteps for its batch and its 512 channels.

Key optimizations over the f32r baseline (629us):
  - per-gate matmul dtypes: i/f gates via fp8e4m3 DoubleRow matmuls (2 k-tiles
    per instruction = 2x PE throughput; the i'/f' normalization suppresses
    their quantization noise), o/cell gates in bf16 (error-sensitive paths).
    Weights scaled by 64 into fp8 range; descale rides the ACT `scale`.
  - the f'=sig(f)/(sig(f)+sig(i)) division moved off the DVE: the 3352ns
    DVE RECIPROCAL becomes r = Exp(-Ln(ssum)) on the ACT engine (ln and exp
    share one activation table; sigmoid phases and ln/exp phases are batched
    per token block so only 2 table loads per block are inserted).
  - elementwise ops run 1024-wide on j-chunk pairs (fixed op overhead halved),
    in bf16 where the error budget allows (2x DVE throughput).
  - recurrence via DVE tensor_tensor_scan in linear space, f32 internal state.
Between layers, channel-half pairs exchange h1 via pairwise AllGather in both
bf16 (for bf16-gate rhs) and fp8 (for DoubleRow rhs), overlapped with compute.

Self-contained: hardcodes shapes; only imports the system concourse repo.
"""
import os
import sys

if '/opt/trn_rl_repo' not in sys.path:
    sys.path.insert(0, '/opt/trn_rl_repo')

import numpy as np

B, S, D = 4, 4096, 1024
NCORES = 8
HALF = D // 2           # channels per core: 512
NCHUNK = HALF // 128    # 4 partition chunks of 128 channels
NKT = D // 128          # 8 contraction k-tiles
NKP = NKT // 2          # 4 contraction k-pairs (fp8 DoubleRow)
TBLK = 512              # token block
NBLK = S // TBLK        # 8 token blocks
GCH = 4 * HALF          # gate channels per core: 2048
SC = 64.0               # fp8 weight scale

QNAMES = ("i", "f", "o", "cell")
# which gate quarters run through fp8 DoubleRow matmuls
FP8_GATES = set(os.environ.get("MINLSTM_FP8", "if"))         # subset of {i,f,o,c}
FP8_GATES = {q for q in QNAMES if q[0] in FP8_GATES}
# which elementwise values are computed/stored in bf16
EW_BF16 = set(os.environ.get("MINLSTM_BF16", "h,so,sg,g,bt,a,sf").split(","))

_CACHE = {}


def _split_multi_waits(nc):
    """This walrus build rejects >1 sync wait per instruction. Hoist extra
    waits onto same-engine NoOps inserted just before; engine-queue program
    order makes this semantically identical."""
    from concourse import mybir
    n = 0
    for fn in nc.m.functions:
        for blk in fn.blocks:
            insts = list(blk.instructions)
            new = []
            changed = False
            for inst in insts:
                si = inst.sync_info
                ow = list(si.on_wait) if si is not None and si.on_wait else []
                if len(ow) > 1:
                    changed = True
                    for w in ow[:-1]:
                        n += 1
                        nop = mybir.InstNoOp(name=f"I-wsplit-{n}", ins=[], outs=[])
                        nop.engine = inst.engine
                        nop.sync_info = mybir.SyncInfo(on_wait=[w], on_update=[])
                        new.append(nop)
                    si.on_wait = [ow[-1]]
                new.append(inst)
            if changed:
                blk.instructions = new
    return n


def _build_nc():
    import concourse.bass as bass
    import concourse.mybir as mybir
    import concourse.tile as tile

    f32 = mybir.dt.float32
    bf16 = mybir.dt.bfloat16
    fp8 = mybir.dt.float8e4
    AF = mybir.ActivationFunctionType
    ALU = mybir.AluOpType
    DR = mybir.MatmulPerfMode.DoubleRow

    def ew(name):
        return bf16 if name in EW_BF16 else f32

    q8list = [q for q in QNAMES if q in FP8_GATES]
    qblist = [q for q in QNAMES if q not in FP8_GATES]
    C8 = len(q8list) * HALF      # fp8 weight columns per core
    C16 = len(qblist) * HALF     # bf16 weight columns per core
    use8 = C8 > 0
    use16 = C16 > 0

    nc = bass.Bass("TRN2", target_bir_lowering=False, debug=False,
                   num_devices=NCORES)

    x8_d = nc.dram_tensor("x8T", [D, S], fp8, kind="ExternalInput").ap() \
        if use8 else None
    xb_d = nc.dram_tensor("xbT", [D, S], bf16, kind="ExternalInput").ap() \
        if use16 else None
    w8_d = [nc.dram_tensor(f"w8_{l}", [NKP * 128, 2 * C8], fp8,
                           kind="ExternalInput").ap() if use8 else None
            for l in range(2)]
    w16_d = [nc.dram_tensor(f"w16_{l}", [NKT * 128, C16], bf16,
                            kind="ExternalInput").ap() if use16 else None
             for l in range(2)]
    ba_d = [nc.dram_tensor(f"b{l}a", [128, 16], f32, kind="ExternalInput").ap()
            for l in range(2)]
    bc_d = [nc.dram_tensor(f"b{l}c", [128, 4], f32, kind="ExternalInput").ap()
            for l in range(2)]
    cp_d = [nc.dram_tensor(f"cp{l}", [128, 4], f32, kind="ExternalInput").ap()
            for l in range(2)]
    h2t_d = nc.dram_tensor("h2t", [HALF, S], bf16, kind="ExternalOutput").ap()

    with tile.TileContext(nc) as tc:
        with tc.tile_pool(name="wp", bufs=1) as wp, \
             tc.tile_pool(name="xkp", bufs=2) as xkp, \
             tc.tile_pool(name="gp", bufs=2) as gp, \
             tc.tile_pool(name="g1", bufs=1) as g1, \
             tc.tile_pool(name="cpool", bufs=1) as cpool, \
             tc.tile_pool(name="psum", bufs=8, space="PSUM") as psum, \
             tc.tile_pool(name="dstage", bufs=2, space="DRAM") as dstage, \
             tc.tile_pool(name="dfull", bufs=8, space="DRAM") as dfull:

            # gathered h1 blocks persist through layer 2
            h1f8 = [dfull.tile([D, TBLK], fp8, tag="h1f8", name=f"h1f8_{t}")
                    for t in range(NBLK)] if use8 else None
            h1fb = [dfull.tile([D, TBLK], bf16, tag="h1fb", name=f"h1fb_{t}")
                    for t in range(NBLK)] if use16 else None

            for l in range(2):
                # fp8 weights load first: the i/f-gate matmuls of the first
                # token block need only these, so the PE starts ~6us earlier;
                # the bf16 weights are deferred until after t0's x tiles.
                w8k = []
                if use8:
                    for kk in range(NKP):
                        wk = wp.tile([128, 2, C8], fp8, tag=f"w8_{kk}",
                                     name=f"w8_{l}_{kk}")
                        nc.sync.dma_start(
                            wk[:],
                            w8_d[l][kk * 128:(kk + 1) * 128, :]
                            .rearrange("p (s c) -> p s c", s=2))
                        w8k.append(wk)
                ba = cpool.tile([128, 16], f32, tag=f"ba{l}", name=f"ba{l}")
                nc.sync.dma_start(ba[:], ba_d[l][:])
                bc = cpool.tile([128, 4], f32, tag=f"bc{l}", name=f"bc{l}")
                nc.sync.dma_start(bc[:], bc_d[l][:])
                cp = cpool.tile([128, 4], f32, tag=f"cp{l}", name=f"cp{l}")
                nc.sync.dma_start(cp[:], cp_d[l][:])

                w16k = []
                carry = [None] * NCHUNK
                # Token blocks are processed in PAIRS: the ACT-engine phases
                # (sigmoid table vs ln/exp table) are emitted grouped across
                # both blocks of a pair, so lower_act inserts only 2 table
                # loads per pair instead of per block (1283ns each).
                pend = []
                for t in range(NBLK):
                    # ---- input tiles for this token block ----
                    x8k = []
                    if use8:
                        for kk in range(NKP):
                            xt = xkp.tile([128, 2, TBLK], fp8, tag=f"x8_{kk}",
                                          name=f"x8_{l}_{t}_{kk}")
                            if l == 0:
                                src = x8_d[kk * 256:(kk + 1) * 256,
                                           t * TBLK:(t + 1) * TBLK]
                            else:
                                src = h1f8[t][kk * 256:(kk + 1) * 256, :]
                            nc.sync.dma_start(
                                xt[:], src.rearrange("(s p) n -> p s n", s=2))
                            x8k.append(xt)
                    xbk = []
                    if use16:
                        for kk in range(NKP):
                            xt = xkp.tile([128, 2, TBLK], bf16, tag=f"xb_{kk}",
                                          name=f"xb_{l}_{t}_{kk}")
                            if l == 0:
                                src = xb_d[kk * 256:(kk + 1) * 256,
                                           t * TBLK:(t + 1) * TBLK]
                            else:
                                src = h1fb[t][kk * 256:(kk + 1) * 256, :]
                            nc.sync.dma_start(
                                xt[:], src.rearrange("(s p) n -> p s n", s=2))
                            xbk.append(xt)

                    if t == 0 and use16:
                        # bf16 weights ride the scalar queue for layer 0 (ACT
                        # is idle at startup, and this runs the ~2MB load in
                        # parallel with the sync queue's x tiles); layer 1
                        # stays on sync (the scalar queue is busy with layer-0
                        # sigmoids until the very end of layer 0)
                        w16k.clear()
                        weng = nc.scalar if l == 0 else nc.sync
                        for kk in range(NKP):
                            wk = wp.tile([128, 2, C16], bf16, tag=f"w16_{kk}",
                                         name=f"w16_{l}_{kk}")
                            weng.dma_start(
                                wk[:],
                                w16_d[l][kk * 256:(kk + 1) * 256, :]
                                .rearrange("(s p) c -> p s c", s=2))
                            w16k.append(wk)

                    if l == 0:
                        if use8:
                            h8own = dstage.tile([HALF, TBLK], fp8, tag="h8own",
                                                name=f"h8own{t}")
                        if use16:
                            hbown = dstage.tile([HALF, TBLK], bf16, tag="hbown",
                                                name=f"hbown{t}")

                    # ---- matmuls into PSUM, per (j, quarter) ----
                    ps = {}
                    for j in range(NCHUNK):
                        for q in QNAMES:
                            p = psum.tile([128, TBLK], f32, tag="ps",
                                          name=f"ps_{q}{l}_{t}_{j}")
                            if q in FP8_GATES:
                                off = q8list.index(q) * HALF + j * 128
                                for kk in range(NKP):
                                    nc.tensor.matmul(
                                        p[:], w8k[kk][:, :, off:off + 128],
                                        x8k[kk][:],
                                        start=(kk == 0), stop=(kk == NKP - 1),
                                        perf_mode=DR)
                            else:
                                off = qblist.index(q) * HALF + j * 128
                                for k in range(NKT):
                                    kk, s = k // 2, k % 2
                                    nc.tensor.matmul(
                                        p[:],
                                        w16k[kk][:, s, off:off + 128],
                                        xbk[kk][:, s, :],
                                        start=(k == 0), stop=(k == NKT - 1))
                            ps[(j, q)] = p

                    # ---- wide (j-pair) elementwise tiles ----
                    # gp (bufs=2): tiles crossing engines between iterations;
                    # g1 (bufs=1): tiles produced+consumed in DVE program order
                    def wtile(pool, nm, dt):
                        return [pool.tile([128, 2, TBLK], dt, tag=f"{nm}{p}",
                                          name=f"{nm}{p}_{l}_{t}")
                                for p in range(2)]

                    sf = wtile(gp, "sf", ew("sf"))
                    si = wtile(gp, "si", ew("sf"))
                    sg = wtile(gp, "sg", ew("sg"))
                    so = wtile(gp, "so", ew("so"))
                    cp5 = wtile(gp, "cp5", ew("sg"))
                    ssum = wtile(gp, "ssum", ew("sf"))
                    lns = wtile(gp, "lns", f32)
                    rr = wtile(gp, "rr", ew("sf"))
                    aa = wtile(g1, "aa", ew("a"))
                    ipr = wtile(g1, "ipr", ew("a"))
                    gg = wtile(gp, "gg", ew("g"))
                    bt = wtile(g1, "bt", ew("bt"))
                    cc = wtile(gp, "cc", ew("c"))
                    hh = wtile(gp, "hh", bf16)
                    h8 = wtile(gp, "h8", fp8) if (l == 0 and use8) else None

                    def scl(q):
                        return 1.0 / SC if q in FP8_GATES else 1.0

                    def bcol(qi, j):
                        return ba[:, qi * NCHUNK + j:qi * NCHUNK + j + 1]

                    # final unpaired layer-2 blocks: interleave the ln/exp of
                    # each j-pair half right after its sigmoids, so the tail's
                    # serial ACT->DVE chain overlaps the other half's work
                    # (costs 2 extra table loads, saves tail latency)
                    fast_tail = (l == 1 and t >= NBLK - 2)

                    # ACT sigmoid phase (one table): per j
                    for j in range(NCHUNK):
                        p, jj = j // 2, j % 2
                        nc.scalar.activation(sf[p][:, jj, :], ps[(j, "f")][:],
                                             AF.Sigmoid, bias=bcol(1, j),
                                             scale=scl("f"))
                        nc.scalar.activation(si[p][:, jj, :], ps[(j, "i")][:],
                                             AF.Sigmoid, bias=bcol(0, j),
                                             scale=scl("i"))
                        nc.scalar.activation(sg[p][:, jj, :], ps[(j, "cell")][:],
                                             AF.Sigmoid, bias=bcol(3, j),
                                             scale=scl("cell"))
                        nc.scalar.activation(so[p][:, jj, :], ps[(j, "o")][:],
                                             AF.Sigmoid, bias=bcol(2, j),
                                             scale=scl("o"))
                        # cp5 = cell/SC + bc + 0.5 on DVE (frees the psum bank)
                        if "cell" in FP8_GATES:
                            nc.vector.tensor_scalar(
                                cp5[p][:, jj, :], ps[(j, "cell")][:],
                                1.0 / SC, bc[:, j:j + 1], ALU.mult, ALU.add)
                        else:
                            nc.vector.tensor_scalar(
                                cp5[p][:, jj, :], ps[(j, "cell")][:],
                                bc[:, j:j + 1], None, ALU.add)
                        if fast_tail and jj == 1:
                            nc.vector.tensor_tensor(ssum[p][:], sf[p][:],
                                                    si[p][:], ALU.add)
                            nc.scalar.activation(lns[p][:], ssum[p][:], AF.Ln)
                            nc.scalar.activation(rr[p][:], lns[p][:], AF.Exp,
                                                 scale=-1.0)

                    # DVE: ssum per pair
                    if not fast_tail:
                        for p in range(2):
                            nc.vector.tensor_tensor(ssum[p][:], sf[p][:],
                                                    si[p][:], ALU.add)

                    pend.append(dict(
                        t=t, sf=sf, si=si, sg=sg, so=so, cp5=cp5, ssum=ssum,
                        lns=lns, rr=rr, aa=aa, ipr=ipr, gg=gg, bt=bt, cc=cc,
                        hh=hh, h8=h8, fast=fast_tail,
                        h8own=h8own if (l == 0 and use8) else None,
                        hbown=hbown if (l == 0 and use16) else None))
                    # batch the ACT table phases across block pairs, EXCEPT
                    # the last two blocks of layer 2: pairing there puts a
                    # full extra block of elementwise latency on the kernel
                    # tail after the final matmul (2 extra table loads are
                    # cheaper than ~15us of un-overlapped tail)
                    unpaired = (l == 1 and t >= NBLK - 2)
                    if len(pend) < 2 and not unpaired:
                        continue

                    # ACT ln then exp phases for the whole pair (ln and exp
                    # live in different act tables in this compiler build, so
                    # group all Lns together then all Exps); fast-tail blocks
                    # emitted theirs inline with the sigmoids already
                    for st in pend:
                        if st["fast"]:
                            continue
                        for p in range(2):
                            nc.scalar.activation(st["lns"][p][:],
                                                 st["ssum"][p][:], AF.Ln)
                    for st in pend:
                        if st["fast"]:
                            continue
                        for p in range(2):
                            nc.scalar.activation(st["rr"][p][:],
                                                 st["lns"][p][:], AF.Exp,
                                                 scale=-1.0)
                    for st in pend:
                        tt = st["t"]

                        def half_chain(p):
                            # DVE: a, ipr, g, bt for one j-pair half
                            nc.vector.tensor_tensor(
                                st["aa"][p][:], st["sf"][p][:],
                                st["rr"][p][:], ALU.mult)
                            nc.vector.tensor_scalar(
                                st["ipr"][p][:], st["aa"][p][:], -1.0, 1.0,
                                ALU.mult, ALU.add)
                            nc.vector.tensor_tensor(
                                st["gg"][p][:], st["cp5"][p][:],
                                st["sg"][p][:], ALU.max)
                            nc.vector.tensor_tensor(
                                st["bt"][p][:], st["ipr"][p][:],
                                st["gg"][p][:], ALU.mult)

                        def half_scans(p):
                            for jj in range(2):
                                j = 2 * p + jj
                                init = cp[:, j:j + 1] if tt == 0 else carry[j]
                                nc.vector.tensor_tensor_scan(
                                    st["cc"][p][:, jj, :],
                                    st["aa"][p][:, jj, :],
                                    st["bt"][p][:, jj, :],
                                    init, ALU.mult, ALU.add)
                                carry[j] = st["cc"][p][:, jj, TBLK - 1:TBLK]

                        def half_h(p):
                            nc.vector.tensor_tensor(
                                st["hh"][p][:], st["so"][p][:],
                                st["cc"][p][:], ALU.mult)
                            if st["h8"] is not None:
                                nc.vector.tensor_tensor(
                                    st["h8"][p][:], st["so"][p][:],
                                    st["cc"][p][:], ALU.mult)

                        if st["fast"]:
                            # tail blocks: complete each half end-to-end so
                            # the final DMA starts as early as possible
                            for p in range(2):
                                half_chain(p)
                                half_scans(p)
                                half_h(p)
                                nc.sync.dma_start(
                                    h2t_d[p * 256:(p + 1) * 256,
                                          tt * TBLK:(tt + 1) * TBLK]
                                    .rearrange("(s q) n -> q s n", s=2),
                                    st["hh"][p][:])
                            continue

                        for p in range(2):
                            half_chain(p)
                        for p in range(2):
                            half_scans(p)
                        for p in range(2):
                            half_h(p)

                        # ---- outputs ----
                        if l == 0:
                            for p in range(2):
                                if use16:
                                    nc.sync.dma_start(
                                        st["hbown"][p * 256:(p + 1) * 256, :]
                                        .rearrange("(s q) n -> q s n", s=2),
                                        st["hh"][p][:])
                                if use8:
                                    nc.sync.dma_start(
                                        st["h8own"][p * 256:(p + 1) * 256, :]
                                        .rearrange("(s q) n -> q s n", s=2),
                                        st["h8"][p][:])
                            if use16:
                                nc.gpsimd.collective_compute(
                                    "AllGather", ALU.bypass,
                                    replica_groups=[[0, 1], [2, 3],
                                                    [4, 5], [6, 7]],
                                    ins=[st["hbown"].opt()],
                                    outs=[h1fb[tt].opt()],
                                )
                            if use8:
                                nc.gpsimd.collective_compute(
                                    "AllGather", ALU.bypass,
                                    replica_groups=[[0, 1], [2, 3],
                                                    [4, 5], [6, 7]],
                                    ins=[st["h8own"].opt()],
                                    outs=[h1f8[tt].opt()],
                                )
                        else:
                            for p in range(2):
                                nc.sync.dma_start(
                                    h2t_d[p * 256:(p + 1) * 256,
                                          tt * TBLK:(tt + 1) * TBLK]
                                    .rearrange("(s q) n -> q s n", s=2),
                                    st["hh"][p][:])
                    pend = []

    _split_multi_waits(nc)
    return nc


def _shard_inputs(x, W0, b0, W1, b1, c0_prev, c1_prev):
    import ml_dtypes
    f8 = ml_dtypes.float8_e4m3
    bf = ml_dtypes.bfloat16

    q8list = [q for q in QNAMES if q in FP8_GATES]
    qblist = [q for q in QNAMES if q not in FP8_GATES]
    qidx = {q: i for i, q in enumerate(QNAMES)}
    C8 = len(q8list) * HALF
    C16 = len(qblist) * HALF

    x = np.asarray(x, dtype=np.float32)
    xT = [np.ascontiguousarray(x[b].T) for b in range(B)]
    x8T = [t.astype(f8) for t in xT] if C8 else None
    xbT = [t.astype(bf) for t in xT] if C16 else None

    per_layer = []
    for (W, bb) in ((W0, b0), (W1, b1)):
        W = np.asarray(W, dtype=np.float32)
        bb = np.asarray(bb, dtype=np.float32)
        halves = []
        for h in range(2):
            if C8:
                w8 = np.empty((NKP, 128, 2, C8), dtype=f8)
                for kk in range(NKP):
                    for s in range(2):
                        rows = np.concatenate(
                            [qidx[q] * D + h * HALF + np.arange(HALF)
                             for q in q8list])
                        blk = W[rows, (2 * kk + s) * 128:(2 * kk + s + 1) * 128]
                        w8[kk, :, s, :] = (blk.T * np.float32(SC)).astype(f8)
                w8 = np.ascontiguousarray(w8.reshape(NKP * 128, 2 * C8))
            else:
                w8 = None
            if C16:
                w16 = np.empty((NKT, 128, C16), dtype=bf)
                for k in range(NKT):
                    rows = np.concatenate(
                        [qidx[q] * D + h * HALF + np.arange(HALF)
                         for q in qblist])
                    blk = W[rows, k * 128:(k + 1) * 128]
                    w16[k] = blk.T.astype(bf)
                w16 = np.ascontiguousarray(w16.reshape(NKT * 128, C16))
            else:
                w16 = None
            rows_all = np.concatenate(
                [q * D + h * HALF + np.arange(HALF) for q in range(4)])
            ba = np.ascontiguousarray(bb[rows_all].reshape(16, 128).T)
            bc = np.ascontiguousarray(ba[:, 12:16] + np.float32(0.5))
            halves.append((w8, w16, ba, bc))
        per_layer.append(halves)

    cps = []
    for cprev in (c0_prev, c1_prev):
        cprev = np.asarray(cprev, dtype=np.float32)
        halves = []
        for b in range(B):
            row = []
            for h in range(2):
                seg = cprev[b, 0, h * HALF:(h + 1) * HALF]
                row.append(np.ascontiguousarray(seg.reshape(4, 128).T))
            halves.append(row)
        cps.append(halves)

    in_maps = []
    for k in range(NCORES):
        b, h = k // 2, k % 2
        m = {}
        if C8:
            m["x8T"] = x8T[b]
        if C16:
            m["xbT"] = xbT[b]
        for l in range(2):
            w8, w16, ba, bc = per_layer[l][h]
            if C8:
                m[f"w8_{l}"] = w8
            if C16:
                m[f"w16_{l}"] = w16
            m[f"b{l}a"] = ba
            m[f"b{l}c"] = bc
            m[f"cp{l}"] = cps[l][b][h]
        in_maps.append(m)
    return in_maps


def _get_nc():
    if "nc" not in _CACHE:
        _CACHE["nc"] = _build_nc()
    return _CACHE["nc"]


def kernel(x, W0, b0, W1, b1, c0_prev, c1_prev):
    from concourse.bass_utils import run_bass_kernel_spmd

    nc = _get_nc()
    in_maps = _shard_inputs(x, W0, b0, W1, b1, c0_prev, c1_prev)
    res = run_bass_kernel_spmd(nc, in_maps, list(range(NCORES)))
    out = np.empty((B, S, D), dtype=np.float32)
    for k in range(NCORES):
        b, h = k // 2, k % 2
        out[b, :, h * HALF:(h + 1) * HALF] = \
            res.results[k]["h2t"].astype(np.float32).T
    return out
